# revision 1
# baseline (speedup 1.0000x reference)
"""BitLinear int2 (ternary-weight) GEMM on 8 NeuronCores.

out[8192, 16384] = (x[8192, 4096] @ w_q[16384, 4096].T) * gamma, fp16 I/O,
fp32 accumulation.  Measured ~1.79 ms/core HW exec = ~97.7% of the
78.6 TFLOP/s per-core fp16 peak (8192 matmuls x 512 cols / 2.4 GHz).

Strategy: tensor-parallel over out_features — each core gets a 2048-row
shard of w_q, x is replicated; host concatenates the 8 output shards.
Both operands are host-transposed so the contraction dim lands on SBUF
partitions with plain (non-xbar) DMAs; x is further host-packed to
[128, NSB, KT, sb] so each superblock load is per-partition contiguous.
The whole 16MB transposed weight shard stays resident in SBUF as
per-(k-slab, o-half) tiles; x streams through in 256-token superblocks
on the ACT HWDGE ring while weights + outputs use the SP ring; K=4096
accumulates in PSUM across 32 matmuls of [128x128] @ [128x512].  The
first superblock interleaves its two t-tiles k-outer across all 8 PSUM
banks so the PE hides the resident-weight fill; the last t-tile runs
o-block-major so its copyback trails by only one block.  gamma is baked
into the PSUM->SBUF copy as an immediate scale on the scalar engine.
"""

import sys

import numpy as np

for _p in ("/opt/trn_rl_repo", "/root/.axon_site/_ro/trn_rl_repo"):
    if _p not in sys.path:
        sys.path.append(_p)

N_CORES = 8
N_TOKENS = 8192
IN_FEATURES = 4096
OUT_FEATURES = 16384
O_SHARD = OUT_FEATURES // N_CORES  # 2048

P = 128          # partitions / matmul contraction tile
FREE = 512       # matmul moving free dim (one PSUM bank of fp32)
SB = 256         # tokens per x superblock (2 t-tiles)


def _build(gamma: float, T: int = N_TOKENS, K: int = IN_FEATURES, O: int = O_SHARD,
           sb: int = SB):
    import concourse.mybir as mybir
    from concourse import bacc
    from concourse.tile import TileContext

    fp16 = mybir.dt.float16
    fp32 = mybir.dt.float32

    KT = K // P        # 32 k-tiles
    NB = O // FREE     # 4 o-blocks per core
    TT = sb // P       # t-tiles per superblock
    NSB = T // sb      # superblocks

    nc = bacc.Bacc("TRN2", target_bir_lowering=False, debug=False,
                   num_devices=N_CORES)
    # x is host-packed to [128, NSB, KT, sb]: per partition, one superblock's
    # slabs are contiguous (16KB runs -> line-rate DMA descriptors).
    xQ_d = nc.dram_tensor("xQ", (P, NSB, KT, sb), fp16, kind="ExternalInput")
    wT_d = nc.dram_tensor("wT", (K, O), fp16, kind="ExternalInput")
    out_d = nc.dram_tensor("out", (T, O), fp16, kind="ExternalOutput")

    XCH = 8 if KT % 8 == 0 else 1  # x DMA chunks per superblock
    KC = KT // XCH                 # k-slabs per chunk

    with TileContext(nc) as tc:
        with tc.tile_pool(name="wpool", bufs=1) as wpool, \
             tc.tile_pool(name="xpool", bufs=2) as xpool, \
             tc.tile_pool(name="opool", bufs=3) as opool, \
             tc.tile_pool(name="psum", bufs=8, space="PSUM") as psum_pool:

            # x loads ride the ACT HWDGE ring; weights + outputs ride the SP
            # ring, so weight slab 0 is not queued behind x transfers.
            # Superblock 1 instead queues on the SP ring behind the weight
            # stream: it isn't needed until ~60us and must not steal HBM
            # bandwidth from the resident-weight fill.
            def load_x(xt, s, eng=None):
                eng = eng or nc.scalar
                for c in range(XCH):
                    eng.dma_start(
                        out=xt[:, c * KC:(c + 1) * KC, :],
                        in_=xQ_d[:, s, c * KC:(c + 1) * KC, :])

            # Superblock 0: only the first-half chunks (needed in the first
            # ~27us) go on the ACT ring now; the second-half chunks are
            # interleaved into the SP weight stream below at their
            # consumption deadlines, so they don't steal HBM bandwidth from
            # the critical early weight fill.
            xts = {}
            xts[0] = xpool.tile([P, KT, sb], fp16, tag="xt", name="xt_0")
            stagger0 = XCH == 8

            def load_x0_chunk(eng, c):
                eng.dma_start(
                    out=xts[0][:, c * KC:(c + 1) * KC, :],
                    in_=xQ_d[:, 0, c * KC:(c + 1) * KC, :])

            if stagger0:
                for c in range(XCH // 2):
                    load_x0_chunk(nc.scalar, c)
            else:
                load_x(xts[0], 0)

            # Resident transposed weights, one tile per (k-slab, o-half) so
            # matmul dependencies are fine-grained: the k-loop of the first
            # superblock paces along the arriving weight stream instead of
            # waiting for the full 16MB.  (Per-(k, o-block) tiles measured
            # strictly worse: +13ns on every matmul from per-tile dep
            # overhead, +129us total.)
            OH = O // 2
            wts = {}
            for k in range(KT):
                for h in range(2):
                    wk = wpool.tile([P, OH], fp16, name=f"wk_{k}_{h}")
                    nc.sync.dma_start(
                        out=wk[:],
                        in_=wT_d[k * P:(k + 1) * P, h * OH:(h + 1) * OH])
                    wts[(k, h)] = wk
                # Second-half x chunks of superblock 0: chunk 4+i lands
                # behind weight slab 12+2i, well before its PE deadline.
                if stagger0 and k >= 12 and k % 2 == 0 and (k - 12) // 2 < 4:
                    load_x0_chunk(nc.sync, 4 + (k - 12) // 2)

            def w_rhs(k, ob):
                off = ob * FREE
                return wts[(k, off // OH)][:, off % OH:off % OH + FREE]

            def copyback(ot, psums, row):
                for ob in range(NB):
                    nc.scalar.mul(
                        out=ot[:, ob * FREE:(ob + 1) * FREE],
                        in_=psums[ob],
                        mul=gamma,
                    )
                nc.sync.dma_start(out=out_d[row:row + P, :], in_=ot)

            for s in range(NSB):
                t0 = s * sb
                if s not in xts:
                    xts[s] = xpool.tile([P, KT, sb], fp16, tag="xt",
                                        name=f"xt_{s}")
                    load_x(xts[s], s, eng=nc.sync if s == 1 else None)
                xt = xts[s]

                if s == 0:
                    # Interleave both t-tiles k-outer: 8 matmuls per weight
                    # slab keeps the PE ahead of the DMA stream during the
                    # resident-weight fill. Uses all 8 PSUM banks.
                    ots = [opool.tile([P, O], fp16, tag="ot", name=f"ot_{s}_{j}")
                           for j in range(TT)]
                    psums = [[psum_pool.tile([P, FREE], fp32, tag="ps",
                                             name=f"ps_{s}_{j}_{ob}")
                              for ob in range(NB)] for j in range(TT)]
                    for k in range(KT):
                        for j in range(TT):
                            lhsT = xt[:, k, j * P:(j + 1) * P]
                            for ob in range(NB):
                                nc.tensor.matmul(
                                    psums[j][ob],
                                    lhsT=lhsT,
                                    rhs=w_rhs(k, ob),
                                    start=(k == 0),
                                    stop=(k == KT - 1),
                                )
                    for j in range(TT):
                        copyback(ots[j], psums[j], t0 + j * P)
                else:
                    for j in range(TT):
                        ot = opool.tile([P, O], fp16, tag="ot",
                                        name=f"ot_{s}_{j}")
                        row = t0 + j * P
                        last = (s == NSB - 1 and j == TT - 1)
                        if last:
                            # o-block-major: each block's copy + store
                            # overlaps the next block's accumulation, so
                            # only one block's epilogue trails the PE.
                            for ob in range(NB):
                                ps = psum_pool.tile(
                                    [P, FREE], fp32, tag="ps",
                                    name=f"ps_{s}_{j}_{ob}")
                                for k in range(KT):
                                    nc.tensor.matmul(
                                        ps,
                                        lhsT=xt[:, k, j * P:(j + 1) * P],
                                        rhs=w_rhs(k, ob),
                                        start=(k == 0),
                                        stop=(k == KT - 1),
                                    )
                                nc.scalar.mul(
                                    out=ot[:, ob * FREE:(ob + 1) * FREE],
                                    in_=ps,
                                    mul=gamma,
                                )
                                nc.sync.dma_start(
                                    out=out_d[row:row + P,
                                              ob * FREE:(ob + 1) * FREE],
                                    in_=ot[:, ob * FREE:(ob + 1) * FREE])
                            continue
                        psums = [psum_pool.tile([P, FREE], fp32, tag="ps",
                                                name=f"ps_{s}_{j}_{ob}")
                                 for ob in range(NB)]
                        for k in range(KT):
                            lhsT = xt[:, k, j * P:(j + 1) * P]
                            for ob in range(NB):
                                nc.tensor.matmul(
                                    psums[ob],
                                    lhsT=lhsT,
                                    rhs=w_rhs(k, ob),
                                    start=(k == 0),
                                    stop=(k == KT - 1),
                                )
                        copyback(ot, psums, row)

    nc.compile()
    return nc


def _run(inputs, trace=False):
    import os

    from concourse.bass_utils import run_bass_kernel_spmd

    if not trace:
        # A stray BASS_TRACE would route run_bass_kernel_spmd into the NTFF
        # hook import, which this container lacks.
        os.environ["BASS_NEVER_TRACE"] = "1"
    else:
        os.environ.pop("BASS_NEVER_TRACE", None)

    x = np.asarray(inputs["x"])
    w = np.asarray(inputs["w_q"])
    gamma = float(np.asarray(inputs["gamma"]).astype(np.float32).reshape(-1)[0])

    # Pack x to [128, NSB, KT, sb]: xQ[p, s, k, t] = x[s*sb + t, k*128 + p]
    KT, NSB = IN_FEATURES // P, N_TOKENS // SB
    xQ = np.ascontiguousarray(
        x.T.reshape(KT, P, NSB, SB).transpose(1, 2, 0, 3))
    nc = _build(gamma)
    in_maps = []
    for c in range(N_CORES):
        wT_c = np.ascontiguousarray(w[c * O_SHARD:(c + 1) * O_SHARD, :].T)
        in_maps.append({"xQ": xQ, "wT": wT_c})

    res = run_bass_kernel_spmd(nc, in_maps, core_ids=list(range(N_CORES)),
                               trace=trace)
    out = np.concatenate(
        [np.asarray(res.results[c]["out"]) for c in range(N_CORES)], axis=1)
    return out.astype(np.float16, copy=False), res


def kernel(**inputs) -> np.ndarray:
    out, _ = _run(inputs, trace=False)
    return out

